# revision 1
# baseline (speedup 1.0000x reference)
"""CapsuleConv2d Trainium2 kernel — 1D Winograd F(4,3) along W.

Math: out[b,o,h,w,i,j] = sum_{ci,kh,kw} W[j,o,ci,kh,kw] * x[b,ci,h+kh-1,w+kw-1,i,0]
i.e. 3x3 pad-1 conv, effective batch (b,i) = 64 images [64,56,56], Cout=256.

Strategy (8 cores, data-parallel over b; 2 b-groups per core):
  - 1D Winograd F(4,3) along w, interpolation points {0,1,-1,2,-1/2,inf}:
    host computes x_wino = B^T x (fp32 -> ship fp16) and W_wino = G W
    (fp16); device multiplies + accumulates over (ci, kh) in fp32 PSUM;
    y_wino ships back fp16; host applies A^T. Halves PE work vs direct
    conv (288 matmul pair-slots of N=392 = 112,896 PE cycles/core).
  - the 4 ic0 capsule images split across partition halves: rows 0-63 =
    ci for i in {0,1}, rows 64-127 = ci for i in {2,3} -> no SBUF x
    duplication; the two row-tiles run concurrently on the PE (measured
    pair spacing = N/2.4GHz, LDWEIGHTS fully hidden).
  - per (b, h-block(14), t): two 2-bank PSUM tiles (Pv: ch=0, Ps: ch=1,
    one bank per (ch,ipair) combo = [14h x 14tau x 2i = 392 fp32]).
    Drains run in PARALLEL on VectorE (Pv) + ScalarE (Ps) -- separate
    psum tiles per engine avoid tile-level dependency serialization --
    casting to fp16 slabs [128, 6t, 2ip, 392] per (b, hblock), shipped
    as single 9408-B-per-partition DMA runs (DMA is packet-rate
    sensitive; big contiguous runs reach the 358 GB/s HBM cap).
  - 24 warmup matmuls during the DMA lead-in trip the PE HAM activity
    window so real matmuls start at 2.4 GHz instead of 1.2.
  - DMA/core: x_wino 5.0 MB + W 0.6 MB in, y_wino 19.3 MB out (~71 us
    at the HBM cap -- the binding roofline; PE ~58 us, DVE/ACT ~48 us).
"""

import sys

if "/opt/trn_rl_repo" not in sys.path:
    sys.path.insert(0, "/opt/trn_rl_repo")

import numpy as np

NCORES = 8
B, C, H, W_, IC0, WC1, O = 16, 64, 56, 56, 4, 4, 64
CO = WC1 * O  # 256
BPC = B // NCORES  # 2 b-groups per core
M, R = 4, 3  # F(4,3)
ALPHA = M + R - 1  # 6 wino comps
NT = W_ // M  # 14 tiles per row
NH = 14  # h rows per block
NHB = H // NH  # 4 blocks
HP = H + 2  # padded h
NFREE = NH * NT * 2  # 392 = matmul N (one i-pair)

# F(4,3) matrices, points {0, 1, -1, 2, -1/2} + inf (exact, from the
# interpolation construction; verified vs direct conv to 4e-15).
_PTS = [0.0, 1.0, -1.0, 2.0, -0.5]


def _wino_mats():
    from fractions import Fraction as F

    pts = [F(0), F(1), F(-1), F(2), F(-1, 2)]
    n = ALPHA
    V = [[F(0)] * n for _ in range(n)]
    for j, a in enumerate(pts):
        for i in range(n):
            V[j][i] = a**i
    V[n - 1][n - 1] = F(1)
    Mx = [row[:] + [F(1) if k == j else F(0) for k in range(n)]
          for j, row in enumerate(V)]
    for col in range(n):
        piv = next(r for r in range(col, n) if Mx[r][col] != 0)
        Mx[col], Mx[piv] = Mx[piv], Mx[col]
        pv = Mx[col][col]
        Mx[col] = [x / pv for x in Mx[col]]
        for r2 in range(n):
            if r2 != col and Mx[r2][col] != 0:
                f = Mx[r2][col]
                Mx[r2] = [x - f * y for x, y in zip(Mx[r2], Mx[col])]
    L = [row[n:] for row in Mx]
    BT = np.array([[float(L[i][j]) for i in range(n)] for j in range(n)],
                  np.float32)
    AT = np.array([[float(pts[j] ** k) if j < n - 1 else float(k == M - 1)
                    for j in range(n)] for k in range(M)], np.float32)
    G = np.array([[float(pts[j] ** i) if j < n - 1 else float(i == R - 1)
                   for i in range(R)] for j in range(n)], np.float32)
    return BT, AT, G


_BT, _AT, _G = _wino_mats()
_COMPILED = None


def _build():
    import concourse.tile as tile
    from concourse import bacc, mybir

    dt = mybir.dt
    nc = bacc.Bacc("TRN2", target_bir_lowering=False, debug=False,
                   num_devices=NCORES)
    x_d = nc.dram_tensor("x", [BPC, 128, HP, ALPHA, NT * 2], dt.float16,
                         kind="ExternalInput").ap()
    w_d = nc.dram_tensor("w", [64, ALPHA, 2, 3, 128], dt.float16,
                         kind="ExternalInput").ap()
    y_d = nc.dram_tensor("y", [BPC, NHB, 2, 128, ALPHA, 2, NFREE], dt.float16,
                         kind="ExternalOutput").ap()

    with tile.TileContext(nc) as tc:
        with (
            tc.tile_pool(name="xp", bufs=1) as xp,
            tc.tile_pool(name="wp", bufs=1) as wp,
            tc.tile_pool(name="op", bufs=4) as op,
            tc.tile_pool(name="pp", bufs=2, space="PSUM") as pp,
        ):
            wt = wp.tile([128, ALPHA, 2, 3, 128], dt.float16)
            nc.sync.dma_start(wt[0:64, :, :, :, :], w_d[:, :, :, :, :])
            nc.sync.dma_start(wt[64:128, :, :, :, :], wt[0:64, :, :, :, :])
            # PE warmup during DMA lead-in: ~5us of dummy matmuls trip the
            # HAM activity window so real matmuls start at 2.4 GHz. The
            # scratch tiles share the Pv/Ps rings.
            wz = wp.tile([128, 512], dt.float16, tag="wz", name="wz")
            nc.vector.memset(wz[:, :], 0.0)
            Pw1 = pp.tile([128, 2, 512], dt.float32, tag="Pv", name="Pv")
            Pw2 = pp.tile([128, 2, 512], dt.float32, tag="Ps", name="Ps")
            for wu in range(24):
                nc.tensor.matmul((Pw1 if wu % 2 else Pw2)[:, (wu // 2) % 2, :],
                                 lhsT=wz[:, 0:128], rhs=wz[:, :],
                                 start=True, stop=True)

            xts = []
            for b in range(BPC):
                xt = xp.tile([128, HP, ALPHA, NT * 2], dt.float16,
                             tag=f"x{b}", name=f"x{b}")
                xts.append(xt)
                bounds = [0, 16, 30, 44, HP]
                for r0, r1 in zip(bounds, bounds[1:]):
                    nc.sync.dma_start(xt[:, r0:r1, :, :],
                                      x_d[b, :, r0:r1, :, :])

            for b in range(BPC):
                xt = xts[b]
                for hb in range(NHB):
                    h0 = NH * hb
                    ov = op.tile([128, ALPHA, 2, NFREE], dt.float16,
                                 tag="ov", name="ov")
                    os_ = op.tile([128, ALPHA, 2, NFREE], dt.float16,
                                  tag="os", name="os")
                    for t in range(ALPHA):
                        Pv = pp.tile([128, 2, 512], dt.float32, tag="Pv",
                                     name="Pv")
                        Ps = pp.tile([128, 2, 512], dt.float32, tag="Ps",
                                     name="Ps")
                        for ch in range(2):
                            Pc = Pv if ch == 0 else Ps
                            for kh in range(3):
                                st, sp = (kh == 0), (kh == 2)
                                nc.tensor.matmul(
                                    Pc[:, 0, 0:NFREE],
                                    lhsT=wt[0:64, t, ch, kh, :],
                                    rhs=xt[0:64, h0 + kh:h0 + kh + NH, t, :],
                                    start=st, stop=sp,
                                )
                                nc.tensor.matmul(
                                    Pc[:, 1, 0:NFREE],
                                    lhsT=wt[64:128, t, ch, kh, :],
                                    rhs=xt[64:128, h0 + kh:h0 + kh + NH, t, :],
                                    start=st, stop=sp,
                                )
                        nc.vector.tensor_copy(ov[:, t, :, :], Pv[:, :, 0:NFREE])
                        nc.scalar.copy(os_[:, t, :, :], Ps[:, :, 0:NFREE])
                        # last block: ship each 3-component half as soon as
                        # its drains land (4704-B runs), shrinking the
                        # end-of-run write flush
                        if b == BPC - 1 and hb == NHB - 1 and t in (2, 5):
                            t0 = t - 2
                            nc.sync.dma_start(
                                y_d[b, hb, 0, :, t0:t + 1, :, :],
                                ov[:, t0:t + 1, :, :])
                            nc.sync.dma_start(
                                y_d[b, hb, 1, :, t0:t + 1, :, :],
                                os_[:, t0:t + 1, :, :])
                    if not (b == BPC - 1 and hb == NHB - 1):
                        nc.sync.dma_start(y_d[b, hb, 0, :, :, :, :],
                                          ov[:, :, :, :])
                        nc.sync.dma_start(y_d[b, hb, 1, :, :, :, :],
                                          os_[:, :, :, :])

    nc.compile()
    return nc


def _prep(x, W):
    x = np.asarray(x, dtype=np.float32)
    W = np.asarray(W, dtype=np.float32)
    xs = x[..., 0]  # [B, C, H, W, IC0]
    # input transform along w: taps d in 0..5 at w = 4*tau + d - 1
    xpw = np.zeros((B, C, H, W_ + 2, IC0), np.float32)
    xpw[:, :, :, 1:W_ + 1, :] = xs
    st = xpw.strides
    dtap = np.lib.stride_tricks.as_strided(
        xpw, (B, C, H, NT, ALPHA, IC0),
        (st[0], st[1], st[2], st[3] * M, st[3], st[4]))
    # xw[b,c,h,t,tau,i] = sum_d BT[t,d] dtap[b,c,h,tau,d,i]
    xw = np.einsum("td,bchudi->bchtui", _BT, dtap, optimize=True)
    # device layout [BPC, 128, HP, ALPHA, NT, 2]; partition p: ci = p % 64,
    # ipair = p // 64, i = ipair*2 + i2; h padded by one zero row each side
    xarr = np.zeros((B, 128, HP, ALPHA, NT * 2), np.float16)
    xwt = xw.astype(np.float16)
    xarr[:, 0:64, 1:H + 1, :, :] = np.ascontiguousarray(
        xwt[:, :, :, :, :, 0:2]).reshape(B, C, H, ALPHA, NT * 2)
    xarr[:, 64:128, 1:H + 1, :, :] = np.ascontiguousarray(
        xwt[:, :, :, :, :, 2:4]).reshape(B, C, H, ALPHA, NT * 2)
    # weights: Wf[co, ci, kh, kw], co = j*64 + o
    Wf = W.reshape(CO, C, 3, 3)
    gw = np.einsum("tk,mckh->tmch", _G, Wf.transpose(0, 1, 3, 2),
                   optimize=True)  # [T, CO, C, KH]
    warr = np.zeros((64, ALPHA, 2, 3, 128), np.float16)
    for chalf in range(2):
        blk = gw[:, chalf * 128:(chalf + 1) * 128]  # [T, 128co, C, KH]
        warr[:, :, chalf, :, :] = blk.transpose(2, 0, 3, 1).astype(np.float16)
    return xarr, warr


def _post(ys):
    # ys: [B, NHB, 2ch, 128, ALPHA, 2ip, NFREE] fp32, per global b
    yw = ys.reshape(B, NHB, 2, 128, ALPHA, 2, NH, NT, 2)
    # indices: [b, hb, ch, co_p, t, ip, h14, tau, i2]
    yw = yw.transpose(0, 2, 3, 1, 6, 7, 5, 8, 4)
    # -> [B, ch, cop, hb, h14, tau, ip, i2, t]
    yw = yw.reshape(B, CO, H, NT, IC0, ALPHA)
    y = np.einsum("pt,bohuit->bohupi", _AT, yw, optimize=True)
    y = y.reshape(B, CO, H, W_, IC0)
    out = (y.reshape(B, WC1, O, H, W_, IC0)
           .transpose(0, 2, 3, 4, 5, 1))
    return np.ascontiguousarray(out, dtype=np.float32)


def _run(x, W, trace=False):
    global _COMPILED
    from concourse.bass_utils import run_bass_kernel_spmd

    if _COMPILED is None:
        _COMPILED = _build()
    nc = _COMPILED
    xarr, warr = _prep(x, W)
    in_maps = [
        {"x": np.ascontiguousarray(xarr[c * BPC:(c + 1) * BPC]), "w": warr}
        for c in range(NCORES)
    ]
    res = run_bass_kernel_spmd(nc, in_maps, core_ids=list(range(NCORES)),
                               trace=trace)
    ys = np.concatenate(
        [np.asarray(res.results[c]["y"], dtype=np.float32)
         for c in range(NCORES)], axis=0)
    return _post(ys), res


def kernel(**inputs) -> np.ndarray:
    return _run(inputs["x"], inputs["W"])[0]



# revision 2
# speedup vs baseline: 1.0289x; 1.0289x over previous
"""CapsuleConv2d Trainium2 kernel — 1D Winograd F(4,3) along W.

Math: out[b,o,h,w,i,j] = sum_{ci,kh,kw} W[j,o,ci,kh,kw] * x[b,ci,h+kh-1,w+kw-1,i,0]
i.e. 3x3 pad-1 conv, effective batch (b,i) = 64 images [64,56,56], Cout=256.

Strategy (8 cores, data-parallel over b; 2 b-groups per core):
  - 1D Winograd F(4,3) along w, interpolation points {0,1,-1,2,-1/2,inf}:
    host computes x_wino = B^T x (fp32 -> ship fp16) and W_wino = G W
    (fp16); device multiplies + accumulates over (ci, kh) in fp32 PSUM;
    y_wino ships back fp16; host applies A^T. Halves PE work vs direct
    conv (288 matmul pair-slots of N=392 = 112,896 PE cycles/core).
  - the 4 ic0 capsule images split across partition halves: rows 0-63 =
    ci for i in {0,1}, rows 64-127 = ci for i in {2,3} -> no SBUF x
    duplication; the two row-tiles run concurrently on the PE (measured
    pair spacing = N/2.4GHz, LDWEIGHTS fully hidden).
  - per (b, h-block(14), t): two 2-bank PSUM tiles (Pv: ch=0, Ps: ch=1,
    one bank per (ch,ipair) combo = [14h x 14tau x 2i = 392 fp32]).
    Drains run in PARALLEL on VectorE (Pv) + ScalarE (Ps) -- separate
    psum tiles per engine avoid tile-level dependency serialization --
    casting to fp16 slabs [128, 6t, 2ip, 392] per (b, hblock), shipped
    as single 9408-B-per-partition DMA runs (DMA is packet-rate
    sensitive; big contiguous runs reach the 358 GB/s HBM cap).
  - 24 warmup matmuls during the DMA lead-in trip the PE HAM activity
    window so real matmuls start at 2.4 GHz instead of 1.2.
  - DMA/core: x_wino 5.0 MB + W 0.6 MB in, y_wino 19.3 MB out (~71 us
    at the HBM cap -- the binding roofline; PE ~58 us, DVE/ACT ~48 us).
"""

import sys

if "/opt/trn_rl_repo" not in sys.path:
    sys.path.insert(0, "/opt/trn_rl_repo")

import numpy as np

NCORES = 8
B, C, H, W_, IC0, WC1, O = 16, 64, 56, 56, 4, 4, 64
CO = WC1 * O  # 256
BPC = B // NCORES  # 2 b-groups per core
M, R = 4, 3  # F(4,3)
ALPHA = M + R - 1  # 6 wino comps
NT = W_ // M  # 14 tiles per row
NH = 14  # h rows per block
NHB = H // NH  # 4 blocks
HP = H + 2  # padded h
NFREE = NH * NT * 2  # 392 = matmul N (one i-pair)

# F(4,3) matrices, points {0, 1, -1, 2, -1/2} + inf (exact, from the
# interpolation construction; verified vs direct conv to 4e-15).
_PTS = [0.0, 1.0, -1.0, 2.0, -0.5]


def _wino_mats():
    from fractions import Fraction as F

    pts = [F(0), F(1), F(-1), F(2), F(-1, 2)]
    n = ALPHA
    V = [[F(0)] * n for _ in range(n)]
    for j, a in enumerate(pts):
        for i in range(n):
            V[j][i] = a**i
    V[n - 1][n - 1] = F(1)
    Mx = [row[:] + [F(1) if k == j else F(0) for k in range(n)]
          for j, row in enumerate(V)]
    for col in range(n):
        piv = next(r for r in range(col, n) if Mx[r][col] != 0)
        Mx[col], Mx[piv] = Mx[piv], Mx[col]
        pv = Mx[col][col]
        Mx[col] = [x / pv for x in Mx[col]]
        for r2 in range(n):
            if r2 != col and Mx[r2][col] != 0:
                f = Mx[r2][col]
                Mx[r2] = [x - f * y for x, y in zip(Mx[r2], Mx[col])]
    L = [row[n:] for row in Mx]
    BT = np.array([[float(L[i][j]) for i in range(n)] for j in range(n)],
                  np.float32)
    AT = np.array([[float(pts[j] ** k) if j < n - 1 else float(k == M - 1)
                    for j in range(n)] for k in range(M)], np.float32)
    G = np.array([[float(pts[j] ** i) if j < n - 1 else float(i == R - 1)
                   for i in range(R)] for j in range(n)], np.float32)
    return BT, AT, G


_BT, _AT, _G = _wino_mats()
_COMPILED = None


def _build():
    import concourse.tile as tile
    from concourse import bacc, mybir

    dt = mybir.dt
    nc = bacc.Bacc("TRN2", target_bir_lowering=False, debug=False,
                   num_devices=NCORES)
    x_d = nc.dram_tensor("x", [BPC, 128, HP, ALPHA, NT * 2], dt.float16,
                         kind="ExternalInput").ap()
    w_d = nc.dram_tensor("w", [64, ALPHA, 2, 3, 128], dt.float16,
                         kind="ExternalInput").ap()
    y_d = nc.dram_tensor("y", [BPC, NHB, 2, 128, ALPHA, 2, NFREE], dt.float16,
                         kind="ExternalOutput").ap()

    with tile.TileContext(nc) as tc:
        with (
            tc.tile_pool(name="xp", bufs=1) as xp,
            tc.tile_pool(name="wp", bufs=1) as wp,
            tc.tile_pool(name="op", bufs=4) as op,
            tc.tile_pool(name="pp", bufs=2, space="PSUM") as pp,
        ):
            xts = []
            for b in range(BPC):
                xt = xp.tile([128, HP, ALPHA, NT * 2], dt.float16,
                             tag=f"x{b}", name=f"x{b}")
                xts.append(xt)
            wt = wp.tile([128, ALPHA, 2, 3, 128], dt.float16)
            # first compute block's x chunk first, then w, then the rest
            nc.sync.dma_start(xts[0][:, 0:16, :, :], x_d[0, :, 0:16, :, :])
            nc.sync.dma_start(wt[0:64, :, :, :, :], w_d[:, :, :, :, :])
            nc.sync.dma_start(wt[64:128, :, :, :, :], wt[0:64, :, :, :, :])
            # PE warmup during DMA lead-in: ~4us of dummy matmuls accumulate
            # the ~3us of continuous PE busy that trips the clock ramp to
            # max p-state, so real matmuls start fast. Scratch tiles share
            # the Pv/Ps rings.
            wz = wp.tile([128, 512], dt.float16, tag="wz", name="wz")
            nc.vector.memset(wz[:, :], 0.0)
            Pw1 = pp.tile([128, 2, 512], dt.float32, tag="Pv", name="Pv")
            Pw2 = pp.tile([128, 2, 512], dt.float32, tag="Ps", name="Ps")
            for wu in range(8):
                nc.tensor.matmul((Pw1 if wu % 2 else Pw2)[:, (wu // 2) % 2, :],
                                 lhsT=wz[:, 0:128], rhs=wz[:, :],
                                 start=True, stop=True)

            for b in range(BPC):
                xt = xts[b]
                bounds = [0, 16, 30, 44, HP] if b else [16, 30, 44, HP]
                for r0, r1 in zip(bounds, bounds[1:]):
                    nc.sync.dma_start(xt[:, r0:r1, :, :],
                                      x_d[b, :, r0:r1, :, :])

            for b in range(BPC):
                xt = xts[b]
                for hb in range(NHB):
                    h0 = NH * hb
                    ov = op.tile([128, ALPHA, 2, NFREE], dt.float16,
                                 tag="ov", name="ov")
                    os_ = op.tile([128, ALPHA, 2, NFREE], dt.float16,
                                  tag="os", name="os")
                    for t in range(ALPHA):
                        Pv = pp.tile([128, 2, 512], dt.float32, tag="Pv",
                                     name="Pv")
                        Ps = pp.tile([128, 2, 512], dt.float32, tag="Ps",
                                     name="Ps")
                        for ch in range(2):
                            Pc = Pv if ch == 0 else Ps
                            for kh in range(3):
                                st, sp = (kh == 0), (kh == 2)
                                nc.tensor.matmul(
                                    Pc[:, 0, 0:NFREE],
                                    lhsT=wt[0:64, t, ch, kh, :],
                                    rhs=xt[0:64, h0 + kh:h0 + kh + NH, t, :],
                                    start=st, stop=sp,
                                )
                                nc.tensor.matmul(
                                    Pc[:, 1, 0:NFREE],
                                    lhsT=wt[64:128, t, ch, kh, :],
                                    rhs=xt[64:128, h0 + kh:h0 + kh + NH, t, :],
                                    start=st, stop=sp,
                                )
                        nc.vector.tensor_copy(ov[:, t, :, :], Pv[:, :, 0:NFREE])
                        nc.scalar.copy(os_[:, t, :, :], Ps[:, :, 0:NFREE])
                        # last block: ship each 3-component half as soon as
                        # its drains land (4704-B runs), shrinking the
                        # end-of-run write flush
                        if b == BPC - 1 and hb == NHB - 1 and t in (2, 5):
                            t0 = t - 2
                            nc.sync.dma_start(
                                y_d[b, hb, 0, :, t0:t + 1, :, :],
                                ov[:, t0:t + 1, :, :])
                            nc.sync.dma_start(
                                y_d[b, hb, 1, :, t0:t + 1, :, :],
                                os_[:, t0:t + 1, :, :])
                    if not (b == BPC - 1 and hb == NHB - 1):
                        nc.sync.dma_start(y_d[b, hb, 0, :, :, :, :],
                                          ov[:, :, :, :])
                        nc.sync.dma_start(y_d[b, hb, 1, :, :, :, :],
                                          os_[:, :, :, :])

    nc.compile()
    return nc


def _prep(x, W):
    x = np.asarray(x, dtype=np.float32)
    W = np.asarray(W, dtype=np.float32)
    xs = x[..., 0]  # [B, C, H, W, IC0]
    # input transform along w: taps d in 0..5 at w = 4*tau + d - 1
    xpw = np.zeros((B, C, H, W_ + 2, IC0), np.float32)
    xpw[:, :, :, 1:W_ + 1, :] = xs
    st = xpw.strides
    dtap = np.lib.stride_tricks.as_strided(
        xpw, (B, C, H, NT, ALPHA, IC0),
        (st[0], st[1], st[2], st[3] * M, st[3], st[4]))
    # xw[b,c,h,t,tau,i] = sum_d BT[t,d] dtap[b,c,h,tau,d,i]
    xw = np.einsum("td,bchudi->bchtui", _BT, dtap, optimize=True)
    # device layout [BPC, 128, HP, ALPHA, NT, 2]; partition p: ci = p % 64,
    # ipair = p // 64, i = ipair*2 + i2; h padded by one zero row each side
    xarr = np.zeros((B, 128, HP, ALPHA, NT * 2), np.float16)
    xwt = xw.astype(np.float16)
    xarr[:, 0:64, 1:H + 1, :, :] = np.ascontiguousarray(
        xwt[:, :, :, :, :, 0:2]).reshape(B, C, H, ALPHA, NT * 2)
    xarr[:, 64:128, 1:H + 1, :, :] = np.ascontiguousarray(
        xwt[:, :, :, :, :, 2:4]).reshape(B, C, H, ALPHA, NT * 2)
    # weights: Wf[co, ci, kh, kw], co = j*64 + o
    Wf = W.reshape(CO, C, 3, 3)
    gw = np.einsum("tk,mckh->tmch", _G, Wf.transpose(0, 1, 3, 2),
                   optimize=True)  # [T, CO, C, KH]
    warr = np.zeros((64, ALPHA, 2, 3, 128), np.float16)
    for chalf in range(2):
        blk = gw[:, chalf * 128:(chalf + 1) * 128]  # [T, 128co, C, KH]
        warr[:, :, chalf, :, :] = blk.transpose(2, 0, 3, 1).astype(np.float16)
    return xarr, warr


def _post(ys):
    # ys: [B, NHB, 2ch, 128, ALPHA, 2ip, NFREE] fp32, per global b
    yw = ys.reshape(B, NHB, 2, 128, ALPHA, 2, NH, NT, 2)
    # indices: [b, hb, ch, co_p, t, ip, h14, tau, i2]
    yw = yw.transpose(0, 2, 3, 1, 6, 7, 5, 8, 4)
    # -> [B, ch, cop, hb, h14, tau, ip, i2, t]
    yw = yw.reshape(B, CO, H, NT, IC0, ALPHA)
    y = np.einsum("pt,bohuit->bohupi", _AT, yw, optimize=True)
    y = y.reshape(B, CO, H, W_, IC0)
    out = (y.reshape(B, WC1, O, H, W_, IC0)
           .transpose(0, 2, 3, 4, 5, 1))
    return np.ascontiguousarray(out, dtype=np.float32)


def _run(x, W, trace=False):
    global _COMPILED
    from concourse.bass_utils import run_bass_kernel_spmd

    if _COMPILED is None:
        _COMPILED = _build()
    nc = _COMPILED
    xarr, warr = _prep(x, W)
    in_maps = [
        {"x": np.ascontiguousarray(xarr[c * BPC:(c + 1) * BPC]), "w": warr}
        for c in range(NCORES)
    ]
    res = run_bass_kernel_spmd(nc, in_maps, core_ids=list(range(NCORES)),
                               trace=trace)
    ys = np.concatenate(
        [np.asarray(res.results[c]["y"], dtype=np.float32)
         for c in range(NCORES)], axis=0)
    return _post(ys), res


def kernel(**inputs) -> np.ndarray:
    return _run(inputs["x"], inputs["W"])[0]



# revision 3
# speedup vs baseline: 1.1232x; 1.0917x over previous
"""CapsuleConv2d Trainium2 kernel — 1D Winograd F(7,3) along W.

Math: out[b,o,h,w,i,j] = sum_{ci,kh,kw} W[j,o,ci,kh,kw] * x[b,ci,h+kh-1,w+kw-1,i,0]
i.e. 3x3 pad-1 conv, effective batch (b,i) = 64 images [64,56,56], Cout=256.

Strategy (8 cores, data-parallel over b; 2 b-groups per core):
  - 1D Winograd F(7,3) along w (56 = 7*8 exactly), points
    {0,+-1,+-2,+-1/2,-4,inf}: host computes x_wino = B^T x (ship fp16)
    and W_wino = G W (fp16); device multiplies + accumulates over
    (ci, kh) in fp32 PSUM; y_wino ships back fp16; host applies A^T.
    End-to-end rel err ~2.5e-3 (gate 2e-2). vs F(4,3): 6/7 the PE work
    AND 6/7 the wino-domain bytes (9 comps per 7 outputs vs 6 per 4).
  - the 4 ic0 capsule images split across partition halves: rows 0-63 =
    ci for i in {0,1}, rows 64-127 = ci for i in {2,3} -> no SBUF x
    duplication; the two row-tiles co-issue on the PE (64-deep
    contraction runs at N/2 cycles; LDWEIGHTS fully hidden).
  - per (b, h-block(14), t): two PSUM tiles (Pv: ch=0, Ps: ch=1), one
    bank per (ch,ip) combo = [14h x 8tau x 2i = 224 fp32]. Drains run
    in PARALLEL on VectorE (Pv) + ScalarE (Ps) -- separate psum tiles
    per engine avoid tile-level dependency serialization -- casting to
    fp16 slabs [128, 9t, 2ip, 224] per (b, hblock), shipped as single
    8064-B-per-partition DMA runs.
  - 8 warmup matmuls during the DMA lead-in accumulate the ~3us of
    continuous PE busy that trips the clock ramp to max p-state.
  - DMA/core: x_wino 4.3 MB + W 0.9 MB in, y_wino 16.5 MB out. The
    binding resource is SDMA engine 15 (serves partitions 92-95/124-127,
    measured ~20% slower per byte than peers): ~64 us for its 1.36 MB
    share. PE ~44 us, DVE/ACT drains ~48 us.
"""

import sys

if "/opt/trn_rl_repo" not in sys.path:
    sys.path.insert(0, "/opt/trn_rl_repo")

import numpy as np

NCORES = 8
B, C, H, W_, IC0, WC1, O = 16, 64, 56, 56, 4, 4, 64
CO = WC1 * O  # 256
BPC = B // NCORES  # 2 b-groups per core
M, R = 7, 3  # F(7,3)
ALPHA = M + R - 1  # 9 wino comps
NT = W_ // M  # 8 tiles per row
NH = 14  # h rows per block
NHB = H // NH  # 4 blocks
HP = H + 2  # padded h
NFREE = NH * NT * 2  # 224 = matmul N (one i-pair)


def _wino_mats():
    from fractions import Fraction as F

    pts = [F(0), F(1), F(-1), F(2), F(-2), F(1, 2), F(-1, 2), F(-4)]
    n = ALPHA
    V = [[F(0)] * n for _ in range(n)]
    for j, a in enumerate(pts):
        for i in range(n):
            V[j][i] = a**i
    V[n - 1][n - 1] = F(1)
    Mx = [row[:] + [F(1) if k == j else F(0) for k in range(n)]
          for j, row in enumerate(V)]
    for col in range(n):
        piv = next(r for r in range(col, n) if Mx[r][col] != 0)
        Mx[col], Mx[piv] = Mx[piv], Mx[col]
        pv = Mx[col][col]
        Mx[col] = [x / pv for x in Mx[col]]
        for r2 in range(n):
            if r2 != col and Mx[r2][col] != 0:
                f = Mx[r2][col]
                Mx[r2] = [x - f * y for x, y in zip(Mx[r2], Mx[col])]
    L = [row[n:] for row in Mx]
    BT = np.array([[float(L[i][j]) for i in range(n)] for j in range(n)],
                  np.float32)
    AT = np.array([[float(pts[j] ** k) if j < n - 1 else float(k == M - 1)
                    for j in range(n)] for k in range(M)], np.float32)
    G = np.array([[float(pts[j] ** i) if j < n - 1 else float(i == R - 1)
                   for i in range(R)] for j in range(n)], np.float32)
    return BT, AT, G


_BT, _AT, _G = _wino_mats()
_COMPILED = None


def _build():
    import concourse.tile as tile
    from concourse import bacc, mybir

    dt = mybir.dt
    nc = bacc.Bacc("TRN2", target_bir_lowering=False, debug=False,
                   num_devices=NCORES)
    x_d = nc.dram_tensor("x", [BPC, 128, HP, ALPHA, NT * 2], dt.float16,
                         kind="ExternalInput").ap()
    w_d = nc.dram_tensor("w", [64, ALPHA, 2, 3, 128], dt.float16,
                         kind="ExternalInput").ap()
    y_d = nc.dram_tensor("y", [BPC, NHB, 2, 128, ALPHA, 2, NFREE], dt.float16,
                         kind="ExternalOutput").ap()

    with tile.TileContext(nc) as tc:
        with (
            tc.tile_pool(name="xp", bufs=1) as xp,
            tc.tile_pool(name="wp", bufs=1) as wp,
            tc.tile_pool(name="op", bufs=4) as op,
            tc.tile_pool(name="pp", bufs=2, space="PSUM") as pp,
        ):
            xts = []
            for b in range(BPC):
                xt = xp.tile([128, HP, ALPHA, NT * 2], dt.float16,
                             tag=f"x{b}", name=f"x{b}")
                xts.append(xt)
            wt = wp.tile([128, ALPHA, 2, 3, 128], dt.float16)
            # first compute block's x chunk first, then w, then the rest
            nc.sync.dma_start(xts[0][:, 0:16, :, :], x_d[0, :, 0:16, :, :])
            nc.sync.dma_start(wt[0:64, :, :, :, :], w_d[:, :, :, :, :])
            nc.sync.dma_start(wt[64:128, :, :, :, :], wt[0:64, :, :, :, :])
            # PE warmup during DMA lead-in: ~4us of dummy matmuls accumulate
            # the ~3us of continuous PE busy that trips the clock ramp to
            # max p-state, so real matmuls start fast. Scratch tiles share
            # the Pv/Ps rings.
            wz = wp.tile([128, 512], dt.float16, tag="wz", name="wz")
            nc.vector.memset(wz[:, :], 0.0)
            Pw1 = pp.tile([128, 2, 512], dt.float32, tag="Pv", name="Pv")
            Pw2 = pp.tile([128, 2, 512], dt.float32, tag="Ps", name="Ps")
            for wu in range(8):
                nc.tensor.matmul((Pw1 if wu % 2 else Pw2)[:, (wu // 2) % 2, :],
                                 lhsT=wz[:, 0:128], rhs=wz[:, :],
                                 start=True, stop=True)

            for b in range(BPC):
                xt = xts[b]
                bounds = [0, 16, 30, 44, HP] if b else [16, 30, 44, HP]
                for r0, r1 in zip(bounds, bounds[1:]):
                    nc.sync.dma_start(xt[:, r0:r1, :, :],
                                      x_d[b, :, r0:r1, :, :])

            for b in range(BPC):
                xt = xts[b]
                for hb in range(NHB):
                    h0 = NH * hb
                    ov = op.tile([128, ALPHA, 2, NFREE], dt.float16,
                                 tag="ov", name="ov")
                    os_ = op.tile([128, ALPHA, 2, NFREE], dt.float16,
                                  tag="os", name="os")
                    for t in range(ALPHA):
                        Pv = pp.tile([128, 2, 512], dt.float32, tag="Pv",
                                     name="Pv")
                        Ps = pp.tile([128, 2, 512], dt.float32, tag="Ps",
                                     name="Ps")
                        for ch in range(2):
                            Pc = Pv if ch == 0 else Ps
                            for kh in range(3):
                                st, sp = (kh == 0), (kh == 2)
                                nc.tensor.matmul(
                                    Pc[:, 0, 0:NFREE],
                                    lhsT=wt[0:64, t, ch, kh, :],
                                    rhs=xt[0:64, h0 + kh:h0 + kh + NH, t, :],
                                    start=st, stop=sp,
                                )
                                nc.tensor.matmul(
                                    Pc[:, 1, 0:NFREE],
                                    lhsT=wt[64:128, t, ch, kh, :],
                                    rhs=xt[64:128, h0 + kh:h0 + kh + NH, t, :],
                                    start=st, stop=sp,
                                )
                        nc.vector.tensor_copy(ov[:, t, :, :], Pv[:, :, 0:NFREE])
                        nc.scalar.copy(os_[:, t, :, :], Ps[:, :, 0:NFREE])
                        # last block: ship each 3-component third as soon as
                        # its drains land, shrinking the end-of-run flush
                        if b == BPC - 1 and hb == NHB - 1 and t in (2, 5, 8):
                            t0 = t - 2
                            nc.sync.dma_start(
                                y_d[b, hb, 0, :, t0:t + 1, :, :],
                                ov[:, t0:t + 1, :, :])
                            nc.sync.dma_start(
                                y_d[b, hb, 1, :, t0:t + 1, :, :],
                                os_[:, t0:t + 1, :, :])
                    if not (b == BPC - 1 and hb == NHB - 1):
                        nc.sync.dma_start(y_d[b, hb, 0, :, :, :, :],
                                          ov[:, :, :, :])
                        nc.sync.dma_start(y_d[b, hb, 1, :, :, :, :],
                                          os_[:, :, :, :])

    nc.compile()
    return nc


def _prep(x, W):
    x = np.asarray(x, dtype=np.float32)
    W = np.asarray(W, dtype=np.float32)
    xs = x[..., 0]  # [B, C, H, W, IC0]
    # input transform along w: taps d in 0..8 at w = 7*tau + d - 1
    xpw = np.zeros((B, C, H, W_ + 2, IC0), np.float32)
    xpw[:, :, :, 1:W_ + 1, :] = xs
    st = xpw.strides
    dtap = np.lib.stride_tricks.as_strided(
        xpw, (B, C, H, NT, ALPHA, IC0),
        (st[0], st[1], st[2], st[3] * M, st[3], st[4]))
    # xw[b,c,h,t,tau,i] = sum_d BT[t,d] dtap[b,c,h,tau,d,i]
    xw = np.einsum("td,bchudi->bchtui", _BT, dtap, optimize=True)
    # device layout [BPC, 128, HP, ALPHA, NT, 2]; partition p: ci = p % 64,
    # ipair = p // 64, i = ipair*2 + i2; h padded by one zero row each side
    xarr = np.zeros((B, 128, HP, ALPHA, NT * 2), np.float16)
    xwt = xw.astype(np.float16)
    xarr[:, 0:64, 1:H + 1, :, :] = np.ascontiguousarray(
        xwt[:, :, :, :, :, 0:2]).reshape(B, C, H, ALPHA, NT * 2)
    xarr[:, 64:128, 1:H + 1, :, :] = np.ascontiguousarray(
        xwt[:, :, :, :, :, 2:4]).reshape(B, C, H, ALPHA, NT * 2)
    # weights: Wf[co, ci, kh, kw], co = j*64 + o
    Wf = W.reshape(CO, C, 3, 3)
    gw = np.einsum("tk,mckh->tmch", _G, Wf.transpose(0, 1, 3, 2),
                   optimize=True)  # [T, CO, C, KH]
    warr = np.zeros((64, ALPHA, 2, 3, 128), np.float16)
    for chalf in range(2):
        blk = gw[:, chalf * 128:(chalf + 1) * 128]  # [T, 128co, C, KH]
        warr[:, :, chalf, :, :] = blk.transpose(2, 0, 3, 1).astype(np.float16)
    return xarr, warr


def _post(ys):
    # ys: [B, NHB, 2ch, 128, ALPHA, 2ip, NFREE] fp32, per global b
    yw = ys.reshape(B, NHB, 2, 128, ALPHA, 2, NH, NT, 2)
    # indices: [b, hb, ch, co_p, t, ip, h14, tau, i2]
    yw = yw.transpose(0, 2, 3, 1, 6, 7, 5, 8, 4)
    # -> [B, ch, cop, hb, h14, tau, ip, i2, t]
    yw = yw.reshape(B, CO, H, NT, IC0, ALPHA)
    y = np.einsum("pt,bohuit->bohupi", _AT, yw, optimize=True)
    y = y.reshape(B, CO, H, W_, IC0)
    out = (y.reshape(B, WC1, O, H, W_, IC0)
           .transpose(0, 2, 3, 4, 5, 1))
    return np.ascontiguousarray(out, dtype=np.float32)


def _run(x, W, trace=False):
    global _COMPILED
    from concourse.bass_utils import run_bass_kernel_spmd

    if _COMPILED is None:
        _COMPILED = _build()
    nc = _COMPILED
    xarr, warr = _prep(x, W)
    in_maps = [
        {"x": np.ascontiguousarray(xarr[c * BPC:(c + 1) * BPC]), "w": warr}
        for c in range(NCORES)
    ]
    res = run_bass_kernel_spmd(nc, in_maps, core_ids=list(range(NCORES)),
                               trace=trace)
    ys = np.concatenate(
        [np.asarray(res.results[c]["y"], dtype=np.float32)
         for c in range(NCORES)], axis=0)
    return _post(ys), res


def kernel(**inputs) -> np.ndarray:
    return _run(inputs["x"], inputs["W"])[0]


# revision 12
# speedup vs baseline: 1.1996x; 1.0680x over previous
"""CapsuleConv2d Trainium2 kernel — 1D Winograd F(7,3) along W.

Math: out[b,o,h,w,i,j] = sum_{ci,kh,kw} W[j,o,ci,kh,kw] * x[b,ci,h+kh-1,w+kw-1,i,0]
i.e. 3x3 pad-1 conv, effective batch (b,i) = 64 images [64,56,56], Cout=256.

Strategy (8 cores, data-parallel over b; 2 b-groups per core):
  - 1D Winograd F(7,3) along w (56 = 7*8 exactly), points
    {0,+-1,+-2,+-1/2,-4,inf}: host computes x_wino = B^T x (ship fp16)
    and W_wino = G W (fp16); device multiplies + accumulates over
    (ci, kh) in fp32 PSUM; y_wino ships back fp16; host applies A^T.
    End-to-end rel err ~2.5e-3 (gate 2e-2). vs F(4,3): 6/7 the PE work
    AND 6/7 the wino-domain bytes (9 comps per 7 outputs vs 6 per 4).
  - the 4 ic0 capsule images split across partition halves: rows 0-63 =
    ci for i in {0,1}, rows 64-127 = ci for i in {2,3} -> no SBUF x
    duplication; the two row-tiles co-issue on the PE (64-deep
    contraction runs at N/2 cycles; LDWEIGHTS fully hidden).
  - per (b, h-block(14), t): two PSUM tiles (Pv: ch=0, Ps: ch=1), one
    bank per (ch,ip) combo = [14h x 8tau x 2i = 224 fp32]. Drains run
    in PARALLEL on VectorE (Pv) + ScalarE (Ps) -- separate psum tiles
    per engine avoid tile-level dependency serialization -- casting to
    fp16 slabs [128, 9t, 2ip, 224] per (b, hblock), shipped as single
    8064-B-per-partition DMA runs.
  - 8 warmup matmuls during the DMA lead-in accumulate the ~3us of
    continuous PE busy that trips the clock ramp to max p-state.
  - DMA/core: x_wino 4.3 MB + W 0.9 MB in, y_wino 16.5 MB out. The
    binding resource is SDMA engine 15 (serves partitions 92-95/124-127,
    measured ~20% slower per byte than peers): ~64 us for its 1.36 MB
    share. PE ~44 us, DVE/ACT drains ~48 us.
"""

import sys

if "/opt/trn_rl_repo" not in sys.path:
    sys.path.insert(0, "/opt/trn_rl_repo")

import numpy as np

NCORES = 8
B, C, H, W_, IC0, WC1, O = 16, 64, 56, 56, 4, 4, 64
CO = WC1 * O  # 256
BPC = B // NCORES  # 2 b-groups per core
M, R = 7, 3  # F(7,3)
ALPHA = M + R - 1  # 9 wino comps
NT = W_ // M  # 8 tiles per row
NH = 14  # h rows per block
NHB = H // NH  # 4 blocks
HP = H + 2  # padded h
NFREE = NH * NT * 2  # 224 = matmul N (one i-pair)


def _wino_mats():
    from fractions import Fraction as F

    pts = [F(0), F(1), F(-1), F(2), F(-2), F(1, 2), F(-1, 2), F(-4)]
    n = ALPHA
    V = [[F(0)] * n for _ in range(n)]
    for j, a in enumerate(pts):
        for i in range(n):
            V[j][i] = a**i
    V[n - 1][n - 1] = F(1)
    Mx = [row[:] + [F(1) if k == j else F(0) for k in range(n)]
          for j, row in enumerate(V)]
    for col in range(n):
        piv = next(r for r in range(col, n) if Mx[r][col] != 0)
        Mx[col], Mx[piv] = Mx[piv], Mx[col]
        pv = Mx[col][col]
        Mx[col] = [x / pv for x in Mx[col]]
        for r2 in range(n):
            if r2 != col and Mx[r2][col] != 0:
                f = Mx[r2][col]
                Mx[r2] = [x - f * y for x, y in zip(Mx[r2], Mx[col])]
    L = [row[n:] for row in Mx]
    BT = np.array([[float(L[i][j]) for i in range(n)] for j in range(n)],
                  np.float32)
    AT = np.array([[float(pts[j] ** k) if j < n - 1 else float(k == M - 1)
                    for j in range(n)] for k in range(M)], np.float32)
    G = np.array([[float(pts[j] ** i) if j < n - 1 else float(i == R - 1)
                   for i in range(R)] for j in range(n)], np.float32)
    return BT, AT, G


_BT, _AT, _G = _wino_mats()
_COMPILED = None


def _build():
    import concourse.tile as tile
    from concourse import bacc, mybir

    dt = mybir.dt
    nc = bacc.Bacc("TRN2", target_bir_lowering=False, debug=False,
                   num_devices=NCORES)
    x_d = nc.dram_tensor("x", [BPC, 128, HP, ALPHA, NT * 2], dt.float16,
                         kind="ExternalInput").ap()
    w_d = nc.dram_tensor("w", [128, ALPHA, 2, 3, 128], dt.float16,
                         kind="ExternalInput").ap()
    y_d = nc.dram_tensor("y", [BPC, NHB, 2, 128, ALPHA, 2, NFREE], dt.float16,
                         kind="ExternalOutput").ap()

    with tile.TileContext(nc) as tc:
        with (
            tc.tile_pool(name="xp", bufs=1) as xp,
            tc.tile_pool(name="wp", bufs=1) as wp,
            tc.tile_pool(name="op", bufs=4) as op,
            tc.tile_pool(name="pp", bufs=2, space="PSUM") as pp,
        ):
            xts = []
            for b in range(BPC):
                xt = xp.tile([128, HP, ALPHA, NT * 2], dt.float16,
                             tag=f"x{b}", name=f"x{b}")
                xts.append(xt)
            wt = wp.tile([128, ALPHA, 2, 3, 128], dt.float16)
            # w + first compute block's x chunk first; b=1's x ships from
            # inside the block loop so output descriptors aren't stuck
            # behind input in the per-engine DMA rings (FIFO per ring).
            nc.sync.dma_start(wt[:, :, :, :, :], w_d[:, :, :, :, :])
            nc.sync.dma_start(xts[0][:, 0:16, :, :], x_d[0, :, 0:16, :, :])
            # PE warmup during DMA lead-in: ~4us of dummy matmuls accumulate
            # the ~3us of continuous PE busy that trips the clock ramp to
            # max p-state, so real matmuls start fast.
            wz = wp.tile([128, 512], dt.float16, tag="wz", name="wz")
            nc.vector.memset(wz[:, :], 0.0)
            Pw1 = pp.tile([128, 2, 512], dt.float32, tag="Pv", name="Pv")
            Pw2 = pp.tile([128, 2, 512], dt.float32, tag="Ps", name="Ps")
            for wu in range(8):
                nc.tensor.matmul((Pw1 if wu % 2 else Pw2)[:, (wu // 2) % 2, :],
                                 lhsT=wz[:, 0:128], rhs=wz[:, :],
                                 start=True, stop=True)

            for r0, r1 in zip([16, 30, 44], [30, 44, HP]):
                nc.sync.dma_start(xts[0][:, r0:r1, :, :],
                                  x_d[0, :, r0:r1, :, :])

            xb1 = [(0, 16), (16, 30), (30, 44), (44, HP)]
            for b in range(BPC):
                xt = xts[b]
                for hb in range(NHB):
                    h0 = NH * hb
                    ov = op.tile([128, ALPHA, 2, NFREE], dt.float16,
                                 tag="ov", name="ov")
                    os_ = op.tile([128, ALPHA, 2, NFREE], dt.float16,
                                  tag="os", name="os")
                    for t in range(ALPHA):
                        Pv = pp.tile([128, 2, 512], dt.float32, tag="Pv",
                                     name="Pv")
                        Ps = pp.tile([128, 2, 512], dt.float32, tag="Ps",
                                     name="Ps")
                        for ch in range(2):
                            Pc = Pv if ch == 0 else Ps
                            for kh in range(3):
                                st, sp = (kh == 0), (kh == 2)
                                nc.tensor.matmul(
                                    Pc[:, 0, 0:NFREE],
                                    lhsT=wt[0:64, t, ch, kh, :],
                                    rhs=xt[0:64, h0 + kh:h0 + kh + NH, t, :],
                                    start=st, stop=sp,
                                )
                                nc.tensor.matmul(
                                    Pc[:, 1, 0:NFREE],
                                    lhsT=wt[64:128, t, ch, kh, :],
                                    rhs=xt[64:128, h0 + kh:h0 + kh + NH, t, :],
                                    start=st, stop=sp,
                                )
                        nc.vector.tensor_copy(ov[:, t, :, :], Pv[:, :, 0:NFREE])
                        nc.scalar.copy(os_[:, t, :, :], Ps[:, :, 0:NFREE])
                        # last block: ship each 3-component third as soon as
                        # its drains land, shrinking the end-of-run flush
                        if b == BPC - 1 and hb == NHB - 1 and t in (2, 5, 8):
                            t0 = t - 2
                            nc.sync.dma_start(
                                y_d[b, hb, 0, :, t0:t + 1, :, :],
                                ov[:, t0:t + 1, :, :])
                            nc.sync.dma_start(
                                y_d[b, hb, 1, :, t0:t + 1, :, :],
                                os_[:, t0:t + 1, :, :])
                    if not (b == BPC - 1 and hb == NHB - 1):
                        nc.sync.dma_start(y_d[b, hb, 0, :, :, :, :],
                                          ov[:, :, :, :])
                        nc.sync.dma_start(y_d[b, hb, 1, :, :, :, :],
                                          os_[:, :, :, :])
                    if b == 0:
                        r0, r1 = xb1[hb]
                        nc.sync.dma_start(xts[1][:, r0:r1, :, :],
                                          x_d[1, :, r0:r1, :, :])

    nc.compile()
    return nc


def _prep(x, W):
    x = np.asarray(x, dtype=np.float32)
    W = np.asarray(W, dtype=np.float32)
    xs = x[..., 0]  # [B, C, H, W, IC0]
    # input transform along w: taps d in 0..8 at w = 7*tau + d - 1
    xpw = np.zeros((B, C, H, W_ + 2, IC0), np.float32)
    xpw[:, :, :, 1:W_ + 1, :] = xs
    st = xpw.strides
    dtap = np.lib.stride_tricks.as_strided(
        xpw, (B, C, H, NT, ALPHA, IC0),
        (st[0], st[1], st[2], st[3] * M, st[3], st[4]))
    # xw[b,c,h,t,tau,i] = sum_d BT[t,d] dtap[b,c,h,tau,d,i]
    xw = np.einsum("td,bchudi->bchtui", _BT, dtap, optimize=True)
    # device layout [BPC, 128, HP, ALPHA, NT, 2]; partition p: ci = p % 64,
    # ipair = p // 64, i = ipair*2 + i2; h padded by one zero row each side
    xarr = np.zeros((B, 128, HP, ALPHA, NT * 2), np.float16)
    xwt = xw.astype(np.float16)
    xarr[:, 0:64, 1:H + 1, :, :] = np.ascontiguousarray(
        xwt[:, :, :, :, :, 0:2]).reshape(B, C, H, ALPHA, NT * 2)
    xarr[:, 64:128, 1:H + 1, :, :] = np.ascontiguousarray(
        xwt[:, :, :, :, :, 2:4]).reshape(B, C, H, ALPHA, NT * 2)
    # weights: Wf[co, ci, kh, kw], co = j*64 + o
    Wf = W.reshape(CO, C, 3, 3)
    gw = np.einsum("tk,mckh->tmch", _G, Wf.transpose(0, 1, 3, 2),
                   optimize=True)  # [T, CO, C, KH]
    warr = np.zeros((128, ALPHA, 2, 3, 128), np.float16)
    for chalf in range(2):
        blk = gw[:, chalf * 128:(chalf + 1) * 128]  # [T, 128co, C, KH]
        warr[0:64, :, chalf, :, :] = blk.transpose(2, 0, 3, 1).astype(np.float16)
    warr[64:128] = warr[0:64]  # pre-duplicated for the upper partition half
    return xarr, warr


def _post(ys):
    # ys: [B, NHB, 2ch, 128, ALPHA, 2ip, NFREE] fp32, per global b
    yw = ys.reshape(B, NHB, 2, 128, ALPHA, 2, NH, NT, 2)
    # indices: [b, hb, ch, co_p, t, ip, h14, tau, i2]
    yw = yw.transpose(0, 2, 3, 1, 6, 7, 5, 8, 4)
    # -> [B, ch, cop, hb, h14, tau, ip, i2, t]
    yw = yw.reshape(B, CO, H, NT, IC0, ALPHA)
    y = np.einsum("pt,bohuit->bohupi", _AT, yw, optimize=True)
    y = y.reshape(B, CO, H, W_, IC0)
    out = (y.reshape(B, WC1, O, H, W_, IC0)
           .transpose(0, 2, 3, 4, 5, 1))
    return np.ascontiguousarray(out, dtype=np.float32)


def _run(x, W, trace=False):
    global _COMPILED
    from concourse.bass_utils import run_bass_kernel_spmd

    if _COMPILED is None:
        _COMPILED = _build()
    nc = _COMPILED
    xarr, warr = _prep(x, W)
    in_maps = [
        {"x": np.ascontiguousarray(xarr[c * BPC:(c + 1) * BPC]), "w": warr}
        for c in range(NCORES)
    ]
    res = run_bass_kernel_spmd(nc, in_maps, core_ids=list(range(NCORES)),
                               trace=trace)
    ys = np.concatenate(
        [np.asarray(res.results[c]["y"], dtype=np.float32)
         for c in range(NCORES)], axis=0)
    return _post(ys), res


def kernel(**inputs) -> np.ndarray:
    return _run(inputs["x"], inputs["W"])[0]
